# revision 1
# baseline (speedup 1.0000x reference)
"""Trainium2 Bass kernel for nn_Attention_5093831213465.

Reference computation (per sample, x_b: [256, 4096]):
  q = Wq @ x_b                       [32, 4096]
  k = maxpool2(Wk @ x_b)             [32, 1024]
  v = maxpool2(Wv @ x_b)             [128, 1024]
  attn = softmax_over_k(k^T @ q)     [1024, 4096]
  out  = Wa @ (v @ attn)             [256, 4096]
  y    = gamma * out + x_b

Sharding: data-parallel over batch, 2 samples per core on 8 cores.

Implementation notes:
- All matmuls run as float32r (full-rate fp32 path on the PE, ~tf32
  precision); x and the weights are declared float32r end-to-end so no
  rounding pass is needed.
- The attention matmuls (contraction dim = 32 channels) are row-packed
  via tile_position at bases (0, 64, 96); walrus requires both
  operands' base partition to equal the row offset, so the conv weight
  is stacked [Wq;Wk;Wq;Wq]: one conv matmul emits q replicated at the
  three pack bases AND the k features (rows 32-63) — M is free on the
  PE, so the k conv costs nothing extra. Pooled k lives at KB[kt%3].
- 2x2 maxpool is a single strided tensor_reduce (max over the two
  innermost window dims) straight out of PSUM.
- Softmax denominator: ones^T @ exp(attn) ridden on the PE, issued
  before the U matmuls so the reciprocal -> partition_broadcast ->
  normalize chain hides under them.
- exp runs PSUM -> SBUF on the scalar engine in [128, 1024] slabs,
  writing float32r directly.
- y stores go out on the Activation engine's HWDGE queue, x loads on
  SP's, to avoid head-of-line blocking between loads and stores.
"""

import sys

import numpy as np

if "/opt/trn_rl_repo" not in sys.path:
    sys.path.insert(0, "/opt/trn_rl_repo")

B, C, H, W = 16, 256, 64, 64
CA = C // 8          # 32  attn channels
CS = C // 2          # 128 value channels
HWF = H * W          # 4096 spatial positions
HWP = HWF // 4       # 1024 pooled positions
SPC = 2              # samples per core
NCORES = 8
CHUNK = 512          # qq columns per chunk
NCHUNK = HWF // CHUNK       # 8
KT = HWP // 128             # 8 kk tiles of 128
KG = 2                      # kk tiles per exp group
NG = KT // KG               # 4 groups
KB = (0, 64, 96)            # row-pack bases (32-63 holds k in the conv psum)

_built = {}


def _build_program():
    from contextlib import ExitStack

    import concourse.bass as bass
    import concourse.tile as tile
    from concourse import bacc, mybir

    f32 = mybir.dt.float32
    f32r = mybir.dt.float32r
    Exp = mybir.ActivationFunctionType.Exp

    nc = bacc.Bacc(
        "TRN2", target_bir_lowering=False, debug=False, enable_asserts=False
    )

    x_d = nc.dram_tensor("x", [SPC, C, HWF], f32r, kind="ExternalInput").ap()
    wq4_d = nc.dram_tensor("wq4T", [C, 128], f32r, kind="ExternalInput").ap()
    wv_d = nc.dram_tensor("wvT", [C, CS], f32r, kind="ExternalInput").ap()
    wa_d = nc.dram_tensor("waTg", [CS, C], f32r, kind="ExternalInput").ap()
    id_d = nc.dram_tensor("ident", [128, 128], f32, kind="ExternalInput").ap()
    on_d = nc.dram_tensor("ones", [128, 1], f32r, kind="ExternalInput").ap()
    y_d = nc.dram_tensor("y", [SPC, C, HWF], f32, kind="ExternalOutput").ap()

    with tile.TileContext(nc) as tc, ExitStack() as ctx:
        consts = ctx.enter_context(tc.tile_pool(name="consts", bufs=1))
        xp = ctx.enter_context(tc.tile_pool(name="xp", bufs=2))
        mid = ctx.enter_context(tc.tile_pool(name="mid", bufs=2))
        ep = ctx.enter_context(tc.tile_pool(name="ep", bufs=6))
        sp = ctx.enter_context(tc.tile_pool(name="sp", bufs=3))
        yp = ctx.enter_context(tc.tile_pool(name="yp", bufs=6))
        psA = ctx.enter_context(tc.tile_pool(name="psA", bufs=2, space="PSUM"))
        psUO = ctx.enter_context(tc.tile_pool(name="psUO", bufs=2, space="PSUM"))
        psS = ctx.enter_context(tc.tile_pool(name="psS", bufs=1, space="PSUM"))
        psC = ctx.enter_context(tc.tile_pool(name="psC", bufs=1, space="PSUM"))

        # weights land in SBUF as f32r directly
        wq4 = consts.tile([128, 2, 128], f32r)
        nc.sync.dma_start(wq4[:], wq4_d.rearrange("(t p) m -> p t m", p=128))
        wv = consts.tile([128, 2, CS], f32r)
        nc.sync.dma_start(wv[:], wv_d.rearrange("(t p) m -> p t m", p=128))
        wa = consts.tile([128, 2, 128], f32r)
        nc.sync.dma_start(wa[:], wa_d.rearrange("p (t m) -> p t m", t=2))
        ident = consts.tile([128, 128], f32)
        nc.sync.dma_start(ident[:], id_d)
        ones = consts.tile([128, 1], f32r)
        nc.sync.dma_start(ones[:], on_d)

        for s in range(SPC):
            xr = xp.tile([128, 2, HWF], f32r, tag="xr")
            for ck in range(NCHUNK):
                cs = slice(ck * CHUNK, (ck + 1) * CHUNK)
                for t in range(2):
                    nc.sync.dma_start(
                        xr[:, t, cs],
                        x_d[s, t * 128 : (t + 1) * 128, cs],
                    )

            # q arrives replicated at the pack bases KB straight from the
            # conv matmul (weight stacked [Wq;Wk;Wq;Wq]); pooled k lives
            # at KB[kt%3] so attention matmuls row-pack via tile_position
            # (walrus: fmap and weight base partition must equal the row
            # tile position). q is split per chunk and v/vT per
            # sample-half so the attention head only waits on the conv
            # work it actually reads (Tile deps are tile-granular).
            qs = []
            kph = mid.tile([128, 3, 128], f32r, name="kph", tag="kp")
            vph = [mid.tile([128, 4, 128], f32, name=f"vph{h}", tag=f"vp{h}") for h in range(2)]
            vTh = [mid.tile([128, 4, 128], f32r, name=f"vTh{h}", tag=f"vT{h}") for h in range(2)]

            # conv (1x1) matmuls + 2x2 maxpool, chunk = 8 rows of h
            for ck in range(NCHUNK):
                cs = slice(ck * CHUNK, (ck + 1) * CHUNK)
                pq4 = psC.tile([128, CHUNK], f32, tag="c")
                for t in range(2):
                    nc.tensor.matmul(
                        pq4[:],
                        wq4[:, t, :],
                        xr[:, t, cs],
                        start=(t == 0),
                        stop=(t == 1),
                    )
                qc = mid.tile([128, CHUNK], f32r, tag=f"q{ck}")
                nc.vector.tensor_copy(qc[:], pq4[:, :])
                qs.append(qc)

                kview = pq4[32:64, :].rearrange(
                    "p (h2 dh w2 dw) -> p h2 w2 dh dw", h2=4, dh=2, w2=32, dw=2
                )
                kb = KB[ck % 3]
                nc.vector.tensor_reduce(
                    kph[kb : kb + 32, ck // 3, :].rearrange(
                        "p (h2 w2) -> p h2 w2", h2=4
                    ),
                    kview,
                    axis=mybir.AxisListType.XY,
                    op=mybir.AluOpType.max,
                )

                pv = psUO.tile([128, CHUNK], f32, tag="uo")
                for t in range(2):
                    nc.tensor.matmul(
                        pv[:],
                        wv[:, t, :],
                        xr[:, t, cs],
                        start=(t == 0),
                        stop=(t == 1),
                    )
                vview = pv[:, :].rearrange(
                    "p (h2 dh w2 dw) -> p h2 w2 dh dw", h2=4, dh=2, w2=32, dw=2
                )
                nc.vector.tensor_reduce(
                    vph[ck // 4][:, ck % 4, :].rearrange(
                        "p (h2 w2) -> p h2 w2", h2=4
                    ),
                    vview,
                    axis=mybir.AxisListType.XY,
                    op=mybir.AluOpType.max,
                )


            # v^T via PE transpose (f32)
            for kt in range(KT):
                ptr = psC.tile([128, 128], f32, tag="c")
                nc.tensor.transpose(ptr[:], vph[kt // 4][:, kt % 4, :], ident[:])
                nc.vector.tensor_copy(vTh[kt // 4][:, kt % 4, :], ptr[:])

            # attention per qq chunk
            for ck in range(NCHUNK):
                cs = slice(ck * CHUNK, (ck + 1) * CHUNK)
                egs = []
                for g in range(NG):
                    pa = psA.tile([128, KG, CHUNK], f32, tag="attn")
                    for j in range(KG):
                        kt = g * KG + j
                        kb = KB[kt % 3]
                        nc.tensor.matmul(
                            pa[:, j, :],
                            kph[kb : kb + 32, kt // 3, :],
                            qs[ck][kb : kb + 32, :],
                            start=True,
                            stop=True,
                            tile_position=(kb, 0),
                        )
                    eg = ep.tile([128, KG, CHUNK], f32r, tag="E")
                    nc.scalar.activation(eg[:], pa[:], Exp)
                    egs.append(eg)

                # s-matmuls first: the recip->broadcast->normalize chain
                # then overlaps the U-matmuls
                psm = psS.tile([1, CHUNK], f32, tag="s")
                for g in range(NG):
                    for j in range(KG):
                        kt = g * KG + j
                        nc.tensor.matmul(
                            psm[:],
                            ones[:],
                            egs[g][:, j, :],
                            start=(kt == 0),
                            stop=(kt == KT - 1),
                        )
                pu = psUO.tile([128, CHUNK], f32, tag="uo")
                for g in range(NG):
                    for j in range(KG):
                        kt = g * KG + j
                        nc.tensor.matmul(
                            pu[:],
                            vTh[kt // 4][:, kt % 4, :],
                            egs[g][:, j, :],
                            start=(kt == 0),
                            stop=(kt == KT - 1),
                        )

                r = sp.tile([1, CHUNK], f32, tag="r")
                nc.vector.reciprocal_approx_fast(r[:], psm[:])
                rb = sp.tile([128, CHUNK], f32, tag="rb")
                nc.gpsimd.partition_broadcast(rb[:], r[0:1, :])
                un = sp.tile([128, CHUNK], f32r, tag="un")
                nc.vector.tensor_mul(un[:], pu[:], rb[:])

                for mt in range(2):
                    po = psUO.tile([128, CHUNK], f32, tag="uo")
                    nc.tensor.matmul(
                        po[:], wa[:, mt, :], un[:], start=True, stop=True
                    )
                    yt = yp.tile([128, CHUNK], f32, tag="y")
                    nc.vector.tensor_add(
                        yt[:], po[:], xr[:, mt, cs].bitcast(f32)
                    )
                    nc.scalar.dma_start(
                        y_d[s, mt * 128 : (mt + 1) * 128, cs], yt[:]
                    )

    nc.compile()
    return nc


def _get_program():
    if "nc" not in _built:
        _built["nc"] = _build_program()
    return _built["nc"]


def _make_in_maps(x, Wq, Wk, Wv, Wa, gamma):
    x = np.ascontiguousarray(np.asarray(x, dtype=np.float32).reshape(B, C, HWF))
    wq4T = np.ascontiguousarray(
        np.concatenate(
            [np.asarray(Wq), np.asarray(Wk), np.asarray(Wq), np.asarray(Wq)],
            axis=0,
        ).T.astype(np.float32)
    )
    wvT = np.ascontiguousarray(np.asarray(Wv).T.astype(np.float32))
    waTg = np.ascontiguousarray(
        (float(np.asarray(gamma).reshape(-1)[0]) * np.asarray(Wa)).T.astype(np.float32)
    )
    ident = np.eye(128, dtype=np.float32)
    ones = np.ones((128, 1), dtype=np.float32)
    return [
        {
            "x": np.ascontiguousarray(x[c * SPC : (c + 1) * SPC]),
            "wq4T": wq4T,
            "wvT": wvT,
            "waTg": waTg,
            "ident": ident,
            "ones": ones,
        }
        for c in range(NCORES)
    ]


def kernel(x, Wq, Wk, Wv, Wa, gamma):
    from concourse import bass_utils

    nc = _get_program()
    in_maps = _make_in_maps(x, Wq, Wk, Wv, Wa, gamma)
    res = bass_utils.run_bass_kernel_spmd(
        nc, in_maps, core_ids=list(range(NCORES))
    )
    out = np.concatenate([res.results[c]["y"] for c in range(NCORES)], axis=0)
    return out.reshape(B, C, H, W)

